# revision 5
# baseline (speedup 1.0000x reference)
"""CVRP decoder kernel for 8 Trainium2 NeuronCores (pure batch data-parallel).

Self-contained: hardcodes shapes B=64,N=256,M=1000,S=500,E=128,H=8,D=16 and
shards the batch 8-per-core.  Key differences vs v1:

- Host-side input prep (in _in_maps): everything the PE/DVE consumes is
  pre-transposed and cast to bf16 on the host so every DMA is dense and
  every matmul streams 1 col/cycle:
    elnT   [E, N]       bf16 (eln transposed)
    kT     [H, D, M]    bf16 (k transposed to head-major, D-major)
    vaug   [M, H, 17]   bf16 (v + ones column per head -> free softmax denom)
    m01T   [M, N]       bf16 multiplicative masks exp(ninf) in [l, n] layout
    m01f   [N, M]       bf16 final-logit multiplicative mask
    wqT/wq_lc/wcT       bf16 pre-transposed weights
- Transposed attention output: att^T[n, (h, d|den)] accumulated in PSUM via
  per-head matmuls with lhsT = u_h[l, n] (the exp'd scores are already in
  that layout) and rhs = vaug_h[l, 17].  272 PE cols per l-tile instead of
  2048, and the softmax denominators ride along as column 16 of each head.
- Combine/final: normalize on DVE (strided recip + broadcast muls), one PE
  transpose of mh, combine and final matmuls all bf16.
- Engine balance: ACT does only exps+tanh (the roofline, one act table);
  DVE does kv mask muls + normalize + PSUM->SBUF copies + final softmax
  muls; Pool does sol mask muls and part of the DMA issue; SP the rest.
  (GPSIMD/Pool cannot access PSUM on this stack.)

Environment workarounds kept from v1: TileContext drain split onto
single-wait NOPs, global one-wait-per-instruction legalization.
"""

import re
from contextlib import ExitStack

import numpy as np

import concourse.bass as bass
import concourse.mybir as mybir
import concourse.tile as tile
from concourse.masks import make_identity

# ---------------------------------------------------------------- constants
B, N, M, S, E, H, D = 64, 256, 1000, 500, 128, 8, 16
SQRT_E = 11.313708498984761
CLIP = 10.0
NINF = -1e9
NCORES = 8
BLOC = B // NCORES  # 8 batch instances per core
VW = 17  # vaug cols per head: 16 v dims + ones column
MP, SP_ = 1024, 512  # mask/vaug row counts padded to full 128-tiles

FP32 = mybir.dt.float32
BF16 = mybir.dt.bfloat16
AF = mybir.ActivationFunctionType

L_TILES = [(i * 128, min(128, M - i * 128)) for i in range((M + 127) // 128)]
L2_TILES = [(i * 128, min(128, S - i * 128)) for i in range((S + 127) // 128)]


# ------------------------------------------------- tile drain-split patch
# This walrus build rejects >1 sync-wait on a Drain ("Too many sync wait
# commands"), so split the kernel-tail global-clock waits onto single-wait
# NOPs preceding the drain.
def _patch_tile_drain():
    from bass_rust import ScopedClock, VectorClock

    def _drain_and_barrier(self, tick_clock, wait_clock):
        gc = tick_clock.global_clock
        vals = [int(x) for x in re.findall(r"\d+", repr(gc))]
        for proc, tick in enumerate(vals):
            if tick > 0:
                partial = VectorClock()
                partial.require_at_least(proc, tick)
                nop = self.nc.sync.nop(nofuse=True, hint="split_drain_wait")
                wait_clock.add_sem_waits(nop.ins, ScopedClock({None: partial}))
        self.nc.sync.drain()  # waits covered by the NOPs above
        self.nc.all_engine_barrier()
        assert self.sems is not None
        popped = self.nc._tile_sem_poison_stack.pop()
        assert popped is self._sem_poison
        self.nc.clear_and_free_semaphores(list(self.sems.allocated().values()))
        self.nc.all_engine_barrier()

    tile.TileContext._drain_and_barrier = _drain_and_barrier


_patch_tile_drain()


def _legalize_single_waits(nc):
    """This walrus build accepts at most ONE sync-wait per instruction; hoist
    extra waits onto single-wait NOP carriers placed just before, on the same
    engine (engines execute in order, so the gate is preserved)."""
    n_multi_upd = 0
    for f in nc.m.functions:
        for bb in f.blocks:
            out = []
            for inst in bb.instructions:
                si = inst.sync_info
                if si is not None and len(si.on_wait) > 1:
                    waits = list(si.on_wait)
                    si.on_wait = waits[-1:]
                    for w in waits[:-1]:
                        nop = mybir.InstNoOp(
                            name=nc.get_next_instruction_name(), ins=[], outs=[])
                        nop.engine = inst.engine
                        nop.sync_info = mybir.SyncInfo(on_wait=[w], on_update=[])
                        out.append(nop)
                if si is not None and len(si.on_update) > 1:
                    n_multi_upd += 1
                out.append(inst)
            bb.instructions = out
    if n_multi_upd:
        print(f"WARNING: {n_multi_upd} instructions with >1 sync updates")


def build_nc(legalize=True):
    nc = bass.Bass(trn_type="TRN2", target_bir_lowering=False, debug=False)

    # DRAM I/O (per-core shard, host-prepped layouts; see _in_maps)
    elnT = nc.dram_tensor("elnT", [BLOC, E, N], BF16, kind="ExternalInput").ap()
    load = nc.dram_tensor("load", [BLOC, N], BF16, kind="ExternalInput").ap()
    kT = nc.dram_tensor("kT", [BLOC, H, D, M], BF16, kind="ExternalInput").ap()
    vaug = nc.dram_tensor("vaug", [BLOC, MP, H, VW], BF16, kind="ExternalInput").ap()
    ksT = nc.dram_tensor("ksT", [BLOC, H, D, S], BF16, kind="ExternalInput").ap()
    vsaug = nc.dram_tensor("vsaug", [BLOC, SP_, H, VW], BF16, kind="ExternalInput").ap()
    m01T = nc.dram_tensor("m01T", [BLOC, MP, N], BF16, kind="ExternalInput").ap()
    s01T = nc.dram_tensor("s01T", [BLOC, SP_, N], BF16, kind="ExternalInput").ap()
    m01f = nc.dram_tensor("m01f", [BLOC, N, M], BF16, kind="ExternalInput").ap()
    shk = nc.dram_tensor("shk", [BLOC, E, M], BF16, kind="ExternalInput").ap()
    wqT = nc.dram_tensor("wqT", [E, E], BF16, kind="ExternalInput").ap()
    wq_lc = nc.dram_tensor("wq_lc", [1, E], BF16, kind="ExternalInput").ap()
    wcT = nc.dram_tensor("wcT", [E, E], BF16, kind="ExternalInput").ap()
    out = nc.dram_tensor("out", [BLOC, N, M], FP32, kind="ExternalOutput").ap()

    with ExitStack() as ctx:
        tc = ctx.enter_context(tile.TileContext(nc))
        build_kernel(ctx, tc, elnT, load, kT, vaug, ksT, vsaug, m01T, s01T,
                     m01f, shk, wqT, wq_lc, wcT, out)
    if legalize:
        _legalize_single_waits(nc)
    return nc


def build_kernel(ctx, tc, elnT_d, load_d, kT_d, vaug_d, ksT_d, vsaug_d,
                 m01T_d, s01T_d, m01f_d, shk_d, wqT_d, wq_lc_d, wcT_d, out_d):
    nc = tc.nc
    ctx.enter_context(nc.allow_low_precision("bf16 matmuls/elementwise"))

    # pools
    singles = ctx.enter_context(tc.tile_pool(name="singles", bufs=1))
    sb_u = ctx.enter_context(tc.tile_pool(name="sb_u", bufs=3))     # exp out
    sb_i = ctx.enter_context(tc.tile_pool(name="sb_i", bufs=2))     # per-inst
    ps_sc = ctx.enter_context(tc.tile_pool(name="ps_sc", bufs=2, space="PSUM"))
    ps_att = ctx.enter_context(tc.tile_pool(name="ps_att", bufs=1, space="PSUM"))
    ps_sm = ctx.enter_context(tc.tile_pool(name="ps_sm", bufs=2, space="PSUM"))

    def small_ps():
        return ps_sm.tile([128, 512], FP32, name="ps", tag="ps")

    # ---------------- once-per-kernel prep ----------------
    # only what instance 0's critical path needs comes first; the rest
    # (identity, wcT, slot-1 memset) is deferred below the startup block
    wqT = singles.tile([E, E], BF16)
    nc.sync.dma_start(out=wqT, in_=wqT_d)
    wq_lc = singles.tile([1, E], BF16)
    nc.sync.dma_start(out=wq_lc, in_=wq_lc_d)
    identf = singles.tile([128, 128], FP32)
    wcT = singles.tile([E, E], BF16)

    # persistent block-diagonal q tiles (zero blocks never rewritten)
    qtz_slots = [singles.tile([128, H * 256], BF16, name=f"qtz{i}",
                              tag=f"qtz{i}") for i in range(2)]

    # ---------------- per batch instance phases ----------------
    def prefetch(b, skip_q=False):
        """Bulk per-instance input DMAs (one DMA each; the 500ns descriptor
        floor makes many small DMAs queue-limiting).  Issued one instance
        ahead so nothing in the compute stream waits on HBM."""
        if not skip_q:
            eln_sb = sb_i.tile([E, N], BF16, tag="eln_sb")
            nc.gpsimd.dma_start(out=eln_sb, in_=elnT_d[b])
            load_sb = sb_i.tile([1, N], BF16, tag="load_sb")
            load_row = bass.AP(tensor=load_d.tensor,
                               offset=load_d.offset + b * N,
                               ap=[[0, 1], [1, N]])
            nc.gpsimd.dma_start(out=load_sb, in_=load_row)
        kfull = sb_i.tile([128, M], BF16, tag="kfull")
        nc.sync.dma_start(out=kfull, in_=kT_d[b])
        ksfull = sb_i.tile([128, S], BF16, tag="ksfull")
        nc.sync.dma_start(out=ksfull, in_=ksT_d[b])
        nkv, nsol = len(L_TILES), len(L2_TILES)
        mkv = sb_i.tile([128, nkv, N], BF16, tag="mkv")
        nc.sync.dma_start(out=mkv, in_=bass.AP(
            tensor=m01T_d.tensor, offset=m01T_d.offset + b * MP * N,
            ap=[[N, 128], [128 * N, nkv], [1, N]]))
        msol = sb_i.tile([128, nsol, N], BF16, tag="msol")
        nc.sync.dma_start(out=msol, in_=bass.AP(
            tensor=s01T_d.tensor, offset=s01T_d.offset + b * SP_ * N,
            ap=[[N, 128], [128 * N, nsol], [1, N]]))
        vakv = sb_i.tile([128, nkv, H * VW], BF16, tag="vakv")
        nc.gpsimd.dma_start(out=vakv, in_=bass.AP(
            tensor=vaug_d.tensor, offset=vaug_d.offset + b * MP * H * VW,
            ap=[[H * VW, 128], [128 * H * VW, nkv], [1, H * VW]]))
        vasol = sb_i.tile([128, nsol, H * VW], BF16, tag="vasol")
        nc.gpsimd.dma_start(out=vasol, in_=bass.AP(
            tensor=vsaug_d.tensor, offset=vsaug_d.offset + b * SP_ * H * VW,
            ap=[[H * VW, 128], [128 * H * VW, nsol], [1, H * VW]]))
        shk_sb = sb_i.tile([128, M], BF16, tag="shk_sb")
        nc.sync.dma_start(out=shk_sb, in_=shk_d[b])
        m01f_sb = sb_i.tile([128, 2, M], BF16, tag="m01f_sb")
        srcf = bass.AP(tensor=m01f_d.tensor, offset=m01f_d.offset + b * N * M,
                       ap=[[M, 128], [128 * M, 2], [1, M]])
        nc.sync.dma_start(out=m01f_sb, in_=srcf)
        if skip_q:
            return ((kfull, mkv, vakv), (ksfull, msol, vasol),
                    shk_sb, m01f_sb)
        return (eln_sb, load_sb, (kfull, mkv, vakv), (ksfull, msol, vasol),
                shk_sb, m01f_sb)

    def qproj(b, eln_sb, load_sb):
        """q projection into the block-diagonal qtz slot (engine-op partition
        bases must be 32-aligned on this stack, so the 16-row head blocks can
        only be spread by DMA)."""
        qT_ps = small_ps()
        nc.tensor.matmul(qT_ps[:, 0:N], wqT, eln_sb, start=True, stop=False)
        nc.tensor.matmul(qT_ps[:, 0:N], wq_lc, load_sb, start=False, stop=True)
        qtz = qtz_slots[b % 2]
        qT = sb_i.tile([128, N], BF16, tag="qT")
        nc.vector.tensor_copy(qT, qT_ps[:, 0:N])
        for h in range(H):
            eng = (nc.sync, nc.gpsimd)[h % 2]
            eng.dma_start(
                out=qtz[16 * h:16 * h + 16, h * 256:(h + 1) * 256],
                in_=qT[16 * h:16 * h + 16, :])

    def mha_stream(b, ltiles, bulk, col0, first, mask_eng, att_ps, tagp):
        """One masked-MHA pass; att^T[n, (h, d|den)] accumulates into att_ps
        (two n-half PSUM tiles) at column base col0."""
        kfull, mfull, vafull = bulk
        qtz = qtz_slots[b % 2]
        nlt = len(ltiles)
        for lt, (l0, L) in enumerate(ltiles):
            # scores for head quads p=0,1 -> exp -> mask -> u
            u = sb_u.tile([128, H, 256], BF16, tag=f"u_{tagp}")
            for p in range(2):
                sc_ps = ps_sc.tile([128, 1024], FP32, tag="sc_ps")
                for j in range(2):
                    nc.tensor.matmul(sc_ps[0:L, j * 512:(j + 1) * 512],
                                     kfull[:, l0:l0 + L],
                                     qtz[:, (4 * p + 2 * j) * 256:
                                         (4 * p + 2 * j + 2) * 256],
                                     start=True, stop=True)
                nc.scalar.activation(u[0:L, 4 * p:4 * p + 4, :], sc_ps[0:L, :],
                                     AF.Exp, scale=0.25)
            mb = bass.AP(tensor=mfull.tensor,
                         offset=mfull.offset + lt * 256,
                         ap=[[mfull.ap[0][0], L], [0, H], [1, 256]])
            mask_eng.tensor_mul(u[0:L], u[0:L], mb)

            # transposed attention accumulation (+ denominator col 16)
            for half in range(2):
                for h in range(H):
                    nc.tensor.matmul(
                        att_ps[half][:, col0 + VW * h:col0 + VW * h + VW],
                        u[0:L, h, half * 128:(half + 1) * 128],
                        vafull[0:L, lt, VW * h:VW * h + VW],
                        start=(first and lt == 0), stop=(lt == nlt - 1),
                        skip_group_check=True)

    def tail_norm(b, att_ps):
        """DVE-only normalize; issued right after the sol stream so the att
        PSUM tiles are freed before the next instance's kv stream needs
        them, without threading the PE through a DVE round-trip."""
        # reciprocals of the 16 denominator cols (stride VW starting at 16)
        mh = sb_i.tile([128, 2, 128], FP32, tag="mh")
        for half in range(2):
            ap_t = att_ps[half]
            den_r = sb_i.tile([128, 16], FP32, tag=f"denr{half}")
            den_src = bass.AP(tensor=ap_t.tensor, offset=ap_t.offset + 16,
                              ap=[[ap_t.ap[0][0], 128], [VW, 16]])
            nc.vector.reciprocal(den_r, den_src)
            # mh = att_kv * denr_kv + att_sol * denr_sol   (per head block)
            t1 = sb_i.tile([128, 128], FP32, tag=f"t1_{half}")
            for c in range(2):
                src = bass.AP(tensor=ap_t.tensor, offset=ap_t.offset + c * 136,
                              ap=[[ap_t.ap[0][0], 128], [VW, H], [1, D]])
                dr = bass.AP(tensor=den_r.tensor, offset=den_r.offset + c * 8,
                             ap=[[den_r.ap[0][0], 128], [1, H], [0, D]])
                if c == 0:
                    nc.vector.tensor_mul(t1, src, dr)
                else:
                    t2 = sb_i.tile([128, 128], FP32, tag=f"t2_{half}")
                    nc.vector.tensor_mul(t2, src, dr)
                    nc.vector.tensor_add(mh[:, half, :], t1, t2)
        return mh

    def tail_rest(b, mh, shk_sb, m01f_sb):
        """transpose -> combine -> final scores/softmax -> out; issued after
        the NEXT instance's kv stream so its PE/ACT/DVE work fills engine
        slack instead of blocking the stream."""
        # transpose mh -> mhT [hd, n] and combine -> cmb [e, n]
        mhT = sb_i.tile([128, 256], BF16, tag="mhT")
        cmb = sb_i.tile([128, 256], BF16, tag="cmb")
        for half in range(2):
            tp = small_ps()
            nc.tensor.transpose(tp[:, 0:128], mh[:, half, :], identf)
            nc.vector.tensor_copy(mhT[:, half * 128:(half + 1) * 128],
                                   tp[:, 0:128])
            cmb_ps = small_ps()
            nc.tensor.matmul(cmb_ps[:, 0:128], wcT,
                             mhT[:, half * 128:(half + 1) * 128],
                             start=True, stop=True)
            nc.vector.tensor_copy(cmb[:, half * 128:(half + 1) * 128],
                                  cmb_ps[:, 0:128])

        # final scores, tanh, exp, mask, row-normalize
        h2 = sb_i.tile([128, 2, M], FP32, tag="h2")
        for nt in range(2):
            t_sb = sb_i.tile([128, M], BF16, tag="t_sb")
            for mt2 in range(2):
                fs_ps = small_ps()
                nc.tensor.matmul(fs_ps[:, 0:500],
                                 cmb[:, nt * 128:(nt + 1) * 128],
                                 shk_sb[:, mt2 * 500:(mt2 + 1) * 500],
                                 start=True, stop=True)
                nc.scalar.activation(t_sb[:, mt2 * 500:(mt2 + 1) * 500],
                                     fs_ps[:, 0:500], AF.Tanh,
                                     scale=float(1.0 / SQRT_E))
            em = sb_i.tile([128, M], BF16, tag="em")
            nc.scalar.activation(em, t_sb, AF.Exp, scale=float(CLIP))
            nc.vector.tensor_mul(em, em, m01f_sb[:, nt, :])
            rs = sb_i.tile([128, 1], FP32, tag="rs")
            nc.vector.tensor_reduce(rs, em, mybir.AxisListType.X,
                                    mybir.AluOpType.add)
            rs_r = sb_i.tile([128, 1], FP32, tag="rs_r")
            nc.vector.reciprocal(rs_r, rs)
            nc.gpsimd.tensor_scalar_mul(h2[:, nt, :], em, rs_r)
            dsto = bass.AP(tensor=out_d.tensor,
                           offset=out_d.offset + (b * 2 + nt) * 128 * M,
                           ap=[[M, 128], [1, M]])
            nc.sync.dma_start(out=dsto, in_=h2[:, nt, :])

    # ---------------- software-pipelined driver ----------------
    def att_tiles():
        return [ps_att.tile([128, 512], FP32, name=f"attT{h}", tag=f"attT{h}")
                for h in range(2)]

    # instance 0: eln/load + q projection first so the qtz DMAs are not
    # queued behind the bulk prefetch on SP; k load next; masks after
    eln0 = sb_i.tile([E, N], BF16, tag="eln_sb")
    nc.gpsimd.dma_start(out=eln0, in_=elnT_d[0])
    load0 = sb_i.tile([1, N], BF16, tag="load_sb")
    nc.gpsimd.dma_start(out=load0, in_=bass.AP(
        tensor=load_d.tensor, offset=load_d.offset, ap=[[0, 1], [1, N]]))
    nc.gpsimd.memset(qtz_slots[0], 0.0)
    qproj(0, eln0, load0)
    pf = (eln0, load0) + prefetch(0, skip_q=True)
    # deferred one-time setup (not on instance 0's critical path)
    nc.vector.memset(qtz_slots[1], 0.0)
    nc.sync.dma_start(out=wcT, in_=wcT_d)
    make_identity(nc, identf)
    rest = None  # (b, mh, shk_sb, m01f_sb) awaiting tail_rest
    first = True
    for b in range(BLOC):
        eln_sb, load_sb, bulk_kv, bulk_sol, shk_sb, m01f_sb = pf
        if not first:
            qproj(b, eln_sb, load_sb)
        first = False
        att_ps = att_tiles()
        mha_stream(b, L_TILES, bulk_kv, 0, True, nc.vector, att_ps, "kv")
        if b + 1 < BLOC:
            pf = prefetch(b + 1)
        if rest is not None:
            tail_rest(*rest)
        mha_stream(b, L2_TILES, bulk_sol, 136, False, nc.gpsimd, att_ps, "sol")
        mh = tail_norm(b, att_ps)
        rest = (b, mh, shk_sb, m01f_sb)
    tail_rest(*rest)


# ------------------------------------------------------------- entry point
_NC_CACHE = None


def kernel(**inputs):
    global _NC_CACHE
    from concourse.bass_utils import run_bass_kernel_spmd

    if _NC_CACHE is None:
        _NC_CACHE = build_nc()
    nc = _NC_CACHE
    res = run_bass_kernel_spmd(nc, _in_maps(inputs), core_ids=list(range(NCORES)))
    return np.concatenate([res.results[c]["out"] for c in range(NCORES)], axis=0)


def _in_maps(inputs):
    bf16 = mybir.dt.np(BF16)
    f32 = np.float32

    eln = np.asarray(inputs["encoded_last_node"], f32)      # [B, N, E]
    load = np.asarray(inputs["load"], f32)                  # [B, N]
    solm = np.asarray(inputs["sols_mask_pomo"], f32)        # [B, N, S]
    ninf = np.asarray(inputs["ninf_mask"], f32)             # [B, N, M]
    k = np.asarray(inputs["k"], f32)                        # [B, H, M, D]
    v = np.asarray(inputs["v"], f32)                        # [B, H, M, D]
    k_s = np.asarray(inputs["k_s"], f32)                    # [B, H, S, D]
    v_s = np.asarray(inputs["v_s"], f32)                    # [B, H, S, D]
    shk = np.asarray(inputs["single_head_key"], f32)        # [B, E, M]
    wq = np.asarray(inputs["Wq_last"], f32)                 # [E, E+1]
    wc = np.asarray(inputs["W_combine"], f32)               # [E, E]

    def aug(vv, LT, LTP):  # [B, H, LT, D] -> [B, LTP, H, VW] with ones col
        out = np.zeros((B, LTP, H, VW), dtype=bf16)
        out[:, :LT, :, :D] = vv.transpose(0, 2, 1, 3).astype(bf16)
        out[:, :LT, :, D] = 1.0
        return out

    def padrows(a, LTP):  # [B, LT, N] -> [B, LTP, N] zero-padded
        out = np.zeros((B, LTP, a.shape[2]), dtype=bf16)
        out[:, :a.shape[1]] = a.astype(bf16)
        return out

    full = {
        "elnT": np.ascontiguousarray(eln.transpose(0, 2, 1)).astype(bf16),
        "load": load.astype(bf16),
        "kT": np.ascontiguousarray(k.transpose(0, 1, 3, 2)).astype(bf16),
        "vaug": aug(v, M, MP),
        "ksT": np.ascontiguousarray(k_s.transpose(0, 1, 3, 2)).astype(bf16),
        "vsaug": aug(v_s, S, SP_),
        "m01T": padrows((ninf == 0.0).transpose(0, 2, 1), MP),
        "s01T": padrows((solm == 0.0).transpose(0, 2, 1), SP_),
        "m01f": (ninf == 0.0).astype(bf16),
        "shk": shk.astype(bf16),
    }
    wqT = np.ascontiguousarray(wq[:, :E].T).astype(bf16)
    wq_lc = np.ascontiguousarray(wq[:, E:].T).astype(bf16)   # [1, E]
    wcT = np.ascontiguousarray(wc.T).astype(bf16)

    in_maps = []
    for c in range(NCORES):
        s = slice(c * BLOC, (c + 1) * BLOC)
        m = {n: np.ascontiguousarray(a[s]) for n, a in full.items()}
        m["wqT"], m["wq_lc"], m["wcT"] = wqT, wq_lc, wcT
        in_maps.append(m)
    return in_maps


def bench(inputs, iters=20):
    """Device-resident repeated execution; returns min wall ns per launch
    (includes PJRT dispatch, excludes H2D of inputs)."""
    import time
    import jax
    import concourse.mybir as mb
    from concourse import bass2jax
    from jax.experimental.shard_map import shard_map
    from jax.sharding import Mesh, NamedSharding, PartitionSpec

    global _NC_CACHE
    if _NC_CACHE is None:
        _NC_CACHE = build_nc()
    nc = _NC_CACHE
    bass2jax.install_neuronx_cc_hook()

    partition_name = nc.partition_id_tensor.name if nc.partition_id_tensor else None
    in_names, out_names, out_avals, zero_outs = [], [], [], []
    for alloc in nc.m.functions[0].allocations:
        if not isinstance(alloc, mb.MemoryLocationSet):
            continue
        name = alloc.memorylocations[0].name
        if alloc.kind == "ExternalInput":
            if name != partition_name:
                in_names.append(name)
        elif alloc.kind == "ExternalOutput":
            shape = tuple(alloc.tensor_shape)
            dtype = mb.dt.np(alloc.dtype)
            out_names.append(name)
            out_avals.append(jax.core.ShapedArray(shape, dtype))
            zero_outs.append(np.zeros((NCORES * shape[0], *shape[1:]), dtype))
    n_params = len(in_names)
    n_outs = len(out_avals)
    all_names = in_names + out_names + ([partition_name] if partition_name else [])
    donate = tuple(range(n_params, n_params + n_outs))

    def _body(*args):
        operands = list(args)
        if partition_name is not None:
            operands.append(bass2jax.partition_id_tensor())
        return tuple(bass2jax._bass_exec_p.bind(
            *operands, out_avals=tuple(out_avals), in_names=tuple(all_names),
            out_names=tuple(out_names), lowering_input_output_aliases=(),
            sim_require_finite=True, sim_require_nnan=True, nc=nc))

    devices = jax.devices()[:NCORES]
    mesh = Mesh(np.asarray(devices), ("core",))
    sharded = jax.jit(
        shard_map(_body, mesh=mesh,
                  in_specs=(PartitionSpec("core"),) * (n_params + n_outs),
                  out_specs=(PartitionSpec("core"),) * n_outs, check_rep=False),
        donate_argnums=donate, keep_unused=True)

    in_maps = _in_maps(inputs)
    concat_in = [np.concatenate([np.asarray(in_maps[c][nm]) for c in range(NCORES)],
                                axis=0) for nm in in_names]
    sh = NamedSharding(mesh, PartitionSpec("core"))
    dev_in = [jax.device_put(a, sh) for a in concat_in]
    # warmup + timed runs; donated zero outputs recreated per iteration
    times = []
    for it in range(iters):
        dev_zeros = [jax.device_put(z, sh) for z in zero_outs]
        jax.block_until_ready(dev_zeros)
        t0 = time.perf_counter()
        outs = sharded(*dev_in, *dev_zeros)
        jax.block_until_ready(outs)
        times.append(time.perf_counter() - t0)
    print(f"  launch times (ms): {[round(t*1e3, 2) for t in times]}")
    return int(min(times[1:]) * 1e9) if len(times) > 1 else int(times[0] * 1e9)


if __name__ == "__main__":
    build_nc()
    print("build ok")
